# revision 24
# baseline (speedup 1.0000x reference)
"""Trainium2 Bass kernel for nn_PointNetLayer (masked per-particle MLP).

Reference computation (per particle, 524288 of them):
    feats = events[..., :16]; mask = events[..., 16]
    h = relu(relu(relu(feats@W1+b1)@W2+b2)@W3+b3)
    out = concat(h, 1) * mask          # (..., 65)

Strategy (pure data parallelism over 8 cores, 65536 particles each):
  - Host pads particle rows 17 -> 32 floats; DMA in contiguous
    [128, 4096] f32 chunks (SWDGE), 128 padded rows per partition.
  - PE transposes [128,128] windows (float32r, 1.5 cyc/row) putting 4
    particle-blocks at 32-aligned partition offsets (features on
    partitions, particles on free dim).
  - 3-layer MLP as block-diagonal-paired float32r matmuls (2 blocks of
    64 features in 128 partitions, free dim 512). Biases are applied in
    the PSUM->SBUF relu copies (per-partition bias APs; ACT for L1/L3,
    DVE 2-op tensor_scalar for L2 to balance engines).
  - PE transposes back to particle-major; mask applied during the
    PSUM->SBUF copy via a 0-stride broadcast AP on DVE; the 65th (mask)
    output column is a small strided copy.
  - DMA out contiguous [128, 8320] f32 chunks (HWDGE).
  - float32r keeps absmax error at ~4e-4 of scale (vs ~3e-3 for the
    optional bf16 paths, which are off by default).
"""

import sys

sys.path.insert(0, "/opt/trn_rl_repo")

import numpy as np

import concourse.bass as bass
import concourse.bacc as bacc
import concourse.mybir as mybir
import concourse.tile as tile
from concourse.ap import AP
from concourse.bass_utils import run_bass_kernel_spmd

F32 = mybir.dt.float32
F32R = mybir.dt.float32r
AF = mybir.ActivationFunctionType
ALU = mybir.AluOpType

B, P, F = 4096, 128, 17
DIN, H, DOUT = 16, 64, 64
NCORES = 8
NPART = B * P // NCORES          # 65536 particles per core
SUP = 2048                        # particles per super-tile (compute unit)
CH_SUP = 4                        # super-tiles per DMA chunk
CHUNK = SUP * CH_SUP              # 8192 particles per DMA
NCH = NPART // CHUNK              # 8 chunks
FOUT = DOUT + 1                   # 65
FP = 32                           # host-padded row length (17 -> 32)

TRACE = False
LAST_RESULTS = None
CFG = None  # optional cfg override used by kernel()

_CACHE = {}


def _inject(ap2d: AP, dims) -> AP:
    """Insert extra free dims between the partition dim and the last free
    dim of a 2-d AP."""
    lst = list(ap2d.ap)
    assert len(lst) == 2, lst
    return AP(ap2d.tensor, ap2d.offset, [lst[0], *[list(d) for d in dims], lst[1]])


DEFAULT_CFG = dict(bf16_x=False, bf16_h3=False, in_dma="swdge",
                   bufs_sb=3, bufs_obuf=3, ch_sup=8, mask1op=False,
                   mcol_pool=False, out_alt=False, xin_bufs=3, out_split=2)


def _build(weights, reps=1, cfg=None):
    cfg = {**DEFAULT_CFG, **(cfg or {})}
    W1, b1, W2, b2, W3, b3 = weights
    nc = bacc.Bacc("TRN2", target_bir_lowering=False, debug=False,
                   num_devices=NCORES)

    ev = nc.dram_tensor("events", [NPART, FP], F32R, kind="ExternalInput")
    out = nc.dram_tensor("out", [NPART, FOUT], F32, kind="ExternalOutput")

    # Host-preprocessed constant layouts (embedded in the NEFF).
    W1P = np.zeros((128, 128), np.float32)
    for r, c in ((0, 0), (32, 64), (64, 0), (96, 64)):
        W1P[r:r + DIN, c:c + H] = W1
    W2P = np.zeros((128, 128), np.float32)
    W2P[0:64, 0:64] = W2
    W2P[64:128, 64:128] = W2
    W3P = np.zeros((128, 128), np.float32)
    W3P[0:64, 0:64] = W3
    W3P[64:128, 64:128] = W3
    I128 = np.eye(128, dtype=np.float32)
    b1b = np.concatenate([b1, b1])[:, None].astype(np.float32)
    b2b = np.concatenate([b2, b2])[:, None].astype(np.float32)
    b3b = np.concatenate([b3, b3])[:, None].astype(np.float32)

    w1d = nc.inline_tensor(W1P, "w1p")
    w2d = nc.inline_tensor(W2P, "w2p")
    w3d = nc.inline_tensor(W3P, "w3p")
    idd = nc.inline_tensor(I128, "i128")
    b1d = nc.inline_tensor(b1b, "b1b")
    b2d = nc.inline_tensor(b2b, "b2b")
    b3d = nc.inline_tensor(b3b, "b3b")

    with tile.TileContext(nc) as tc:
        _kernel_body(tc, ev, out, w1d, w2d, w3d, idd, b1d, b2d, b3d, reps, cfg)

    nc.compile()
    return nc


def _kernel_body(tc, ev, out, w1d, w2d, w3d, idd, b1d, b2d, b3d, reps, cfg):
    CH_SUP = cfg["ch_sup"]
    CHUNK = SUP * CH_SUP
    NCH = NPART // CHUNK
    BF16 = mybir.dt.bfloat16
    dt_x = BF16 if cfg["bf16_x"] else F32R
    dt_h3 = BF16 if cfg["bf16_h3"] else F32R
    nc = tc.nc
    from contextlib import ExitStack
    ctx = ExitStack()

    cpool = ctx.enter_context(tc.tile_pool(name="consts", bufs=1))
    w1f = cpool.tile([128, 128], F32, name="w1f")
    w2f = cpool.tile([128, 128], F32, name="w2f")
    w3f = cpool.tile([128, 128], F32, name="w3f")
    i1f = cpool.tile([128, 128], F32, name="i1f")
    w1s = cpool.tile([128, 128], dt_x, name="w1s")
    w2s = cpool.tile([128, 128], F32R, name="w2s")
    w3s = cpool.tile([128, 128], F32R, name="w3s")
    i128 = cpool.tile([128, 128], dt_x, name="i128")
    i128b = cpool.tile([128, 128], dt_h3, name="i128b")
    b1s = cpool.tile([128, 1], F32, name="b1s")
    b2s = cpool.tile([128, 1], F32, name="b2s")
    b3s = cpool.tile([128, 1], F32, name="b3s")
    for t, d in ((w1f, w1d), (w2f, w2d), (w3f, w3d), (i1f, idd),
                 (b1s, b1d), (b2s, b2d), (b3s, b3d)):
        nc.sync.dma_start(t[:], d.ap())
    for dst, srct in ((w1s, w1f), (w2s, w2f), (w3s, w3f), (i128, i1f),
                      (i128b, i1f)):
        nc.vector.tensor_copy(dst[:], srct[:])

    bs = cfg["bufs_sb"]
    xpool = ctx.enter_context(tc.tile_pool(name="xin", bufs=cfg["xin_bufs"]))
    xtpool = ctx.enter_context(tc.tile_pool(name="xts", bufs=bs))
    h1pool = ctx.enter_context(tc.tile_pool(name="h1s", bufs=bs))
    h2pool = ctx.enter_context(tc.tile_pool(name="h2s", bufs=bs))
    h3pool = ctx.enter_context(tc.tile_pool(name="h3s", bufs=bs))
    opool = ctx.enter_context(tc.tile_pool(name="obuf", bufs=cfg["bufs_obuf"]))
    tinp = ctx.enter_context(tc.tile_pool(name="tinp", bufs=2, space="PSUM"))
    mmp = ctx.enter_context(tc.tile_pool(name="mmp", bufs=3, space="PSUM"))

    for c in [i for _ in range(reps) for i in range(NCH)]:
        # ---- load [128, 2048] contiguous (64 padded rows per partition)
        xin = xpool.tile([128, CH_SUP * 16 * FP], dt_x, tag="xin")
        src = AP(ev, c * CHUNK * FP, [[CH_SUP * 16 * FP, 128],
                                      [1, CH_SUP * 16 * FP]])
        if cfg["bf16_x"] or cfg["in_dma"] == "swdge":
            nc.gpsimd.dma_start(xin[:], src)
        else:
            nc.scalar.dma_start(xin[:], src)
        obuf = opool.tile([128, CH_SUP * 16 * FOUT], F32, tag="obuf")

        for s in range(CH_SUP):
            xv = xin[:, 512 * s:512 * (s + 1)]
            ov = obuf[:, 1040 * s:1040 * (s + 1)]

            # ---- transpose in: 4 x ([128, 128] window -> [128, 128])
            tpsum = tinp.tile([128, 512], dt_x, tag="tps")
            for t in range(4):
                nc.tensor.transpose(tpsum[:, 128 * t:128 * (t + 1)],
                                    xv[:, 128 * t:128 * (t + 1)], i128[:])

            xts = xtpool.tile([128, 512], dt_x, tag="xts")
            nc.scalar.copy(xts[:], tpsum[:])

            # ---- L1: blockdiag pairs, K=49, N=512
            ps1 = mmp.tile([128, 1024], F32, tag="mm")
            nc.tensor.matmul(ps1[:, 0:512], w1s[0:49, :], xts[0:49, :],
                             start=True, stop=True)
            nc.tensor.matmul(ps1[:, 512:1024], w1s[64:113, :], xts[64:113, :],
                             start=True, stop=True)
            h1s = h1pool.tile([128, 1024], F32R, tag="h1s")
            nc.scalar.activation(h1s[:], ps1[:], AF.Relu, bias=b1s[:])

            # ---- L2
            ps2 = mmp.tile([128, 1024], F32, tag="mm")
            nc.tensor.matmul(ps2[:, 0:512], w2s[:], h1s[:, 0:512],
                             start=True, stop=True)
            nc.tensor.matmul(ps2[:, 512:1024], w2s[:], h1s[:, 512:1024],
                             start=True, stop=True)
            h2s = h2pool.tile([128, 1024], F32R, tag="h2s")
            nc.vector.tensor_scalar(h2s[:], ps2[:], b2s[:], 0.0,
                                    ALU.add, ALU.max)

            # ---- L3
            ps3 = mmp.tile([128, 1024], F32, tag="mm")
            nc.tensor.matmul(ps3[:, 0:512], w3s[:], h2s[:, 0:512],
                             start=True, stop=True)
            nc.tensor.matmul(ps3[:, 512:1024], w3s[:], h2s[:, 512:1024],
                             start=True, stop=True)
            h3s = h3pool.tile([128, 1024], dt_h3, tag="h3s")
            nc.scalar.activation(h3s[:], ps3[:], AF.Relu, bias=b3s[:])

            # ---- transpose out: 8 x [128,128] -> particle-major pairs
            psb = mmp.tile([128, 1024], dt_h3, tag="mm")
            for u in range(8):
                nc.tensor.transpose(psb[:, 128 * u:128 * (u + 1)],
                                    h3s[:, 128 * u:128 * (u + 1)], i128b[:])

            # ---- masked copy into output buffer + mask column
            # column groups: half h=0 -> blocks (4t, 4t+1); h=1 -> (4t+2, 4t+3)
            if cfg["mask1op"]:
                d0 = ov[:, 0:64]
                dst = AP(d0.tensor, d0.offset,
                         [list(d0.ap[0]), [130, 2], [260, 4], [65, 2], [1, 64]])
                m0 = xv[:, 16:17]
                msk = AP(m0.tensor, m0.offset,
                         [list(m0.ap[0]), [64, 2], [128, 4], [32, 2], [0, 64]])
                nc.vector.tensor_tensor(dst, psb[:], msk, ALU.mult)
            else:
                for h in range(2):
                    dst = _inject(ov[:, 130 * h:130 * h + 64],
                                  [[260, 4], [65, 2]])
                    msk = xv[:, 16 + 64 * h:16 + 64 * h + 1]
                    msk = AP(msk.tensor, msk.offset,
                             [list(msk.ap[0]), [128, 4], [32, 2], [0, 64]])
                    nc.vector.tensor_tensor(dst, psb[:, 512 * h:512 * (h + 1)],
                                            msk, ALU.mult)
            mcol_dst = _inject(ov[:, 64:65], [[65, 16]])
            mcol_src = _inject(xv[:, 16:17], [[32, 16]])
            if cfg["mcol_pool"]:
                nc.gpsimd.tensor_copy(mcol_dst, mcol_src)
            else:
                nc.vector.tensor_copy(mcol_dst, mcol_src)

        oeng = nc.scalar if (cfg["out_alt"] and c % 2 == 1) else nc.sync
        osp = cfg["out_split"]
        sub = CH_SUP // osp
        for j in range(osp):
            # partition p holds rows c*CHUNK + p*(CH_SUP*16) + [0, CH_SUP*16);
            # sub-chunk j is the per-partition column slice [j*sub*16, ...)
            dst = AP(out, c * CHUNK * FOUT + j * sub * 16 * FOUT,
                     [[CH_SUP * 16 * FOUT, 128], [1, sub * 16 * FOUT]])
            oeng.dma_start(dst, obuf[:, j * sub * 1040:(j + 1) * sub * 1040])

    ctx.close()


def kernel(events, W1, b1, W2, b2, W3, b3):
    global LAST_RESULTS
    events = np.ascontiguousarray(np.asarray(events, dtype=np.float32))
    ws = tuple(np.ascontiguousarray(np.asarray(a, np.float32))
               for a in (W1, b1, W2, b2, W3, b3))
    key = tuple(a.tobytes() for a in ws)  # weights are baked into the NEFF
    if _CACHE.get("key") != key:
        _CACHE["nc"] = _build(ws, cfg=CFG)
        _CACHE["key"] = key
    nc = _CACHE["nc"]

    flat = events.reshape(B * P, F)
    padded = np.zeros((B * P, FP), dtype=np.float32)
    padded[:, :F] = flat
    in_maps = [{"events": np.ascontiguousarray(padded[c * NPART:(c + 1) * NPART])}
               for c in range(NCORES)]
    res = run_bass_kernel_spmd(nc, in_maps, core_ids=list(range(NCORES)),
                               trace=TRACE)
    LAST_RESULTS = res
    out = np.concatenate([res.results[c]["out"] for c in range(NCORES)], axis=0)
    return out.reshape(B, P, FOUT)


# revision 26
# speedup vs baseline: 7.5021x; 7.5021x over previous
"""Trainium2 Bass kernel for nn_PointNetLayer (masked per-particle MLP).

Reference computation (per particle, 524288 of them):
    feats = events[..., :16]; mask = events[..., 16]
    h = relu(relu(relu(feats@W1+b1)@W2+b2)@W3+b3)
    out = concat(h, 1) * mask          # (..., 65)

Strategy (pure data parallelism over 8 cores, 65536 particles each):
  - Host pads particle rows 17 -> 32 floats; DMA in contiguous
    [128, 4096] f32 chunks (SWDGE), 128 padded rows per partition.
  - PE transposes [128,128] windows (float32r, 1.5 cyc/row) putting 4
    particle-blocks at 32-aligned partition offsets (features on
    partitions, particles on free dim).
  - 3-layer MLP as block-diagonal-paired float32r matmuls (2 blocks of
    64 features in 128 partitions, free dim 512). Biases are applied in
    the PSUM->SBUF relu copies (per-partition bias APs; ACT for L1/L3,
    DVE 2-op tensor_scalar for L2 to balance engines).
  - PE transposes back to particle-major; mask applied during the
    PSUM->SBUF copy via a 0-stride broadcast AP on DVE; the 65th (mask)
    output column is a small strided copy.
  - DMA out contiguous [128, 8320] f32 chunks (HWDGE).
  - float32r keeps absmax error at ~4e-4 of scale (vs ~3e-3 for the
    optional bf16 paths, which are off by default).
"""

import sys

sys.path.insert(0, "/opt/trn_rl_repo")

import numpy as np

import concourse.bass as bass
import concourse.bacc as bacc
import concourse.mybir as mybir
import concourse.tile as tile
from concourse.ap import AP
from concourse.bass_utils import run_bass_kernel_spmd

F32 = mybir.dt.float32
F32R = mybir.dt.float32r
AF = mybir.ActivationFunctionType
ALU = mybir.AluOpType

B, P, F = 4096, 128, 17
DIN, H, DOUT = 16, 64, 64
NCORES = 8
NPART = B * P // NCORES          # 65536 particles per core
SUP = 2048                        # particles per super-tile (compute unit)
CH_SUP = 4                        # super-tiles per DMA chunk
CHUNK = SUP * CH_SUP              # 8192 particles per DMA
NCH = NPART // CHUNK              # 8 chunks
FOUT = DOUT + 1                   # 65
FP = 32                           # host-padded row length (17 -> 32)

TRACE = False
LAST_RESULTS = None
CFG = None  # optional cfg override used by kernel()

_CACHE = {}


def _inject(ap2d: AP, dims) -> AP:
    """Insert extra free dims between the partition dim and the last free
    dim of a 2-d AP."""
    lst = list(ap2d.ap)
    assert len(lst) == 2, lst
    return AP(ap2d.tensor, ap2d.offset, [lst[0], *[list(d) for d in dims], lst[1]])


DEFAULT_CFG = dict(bf16_x=False, bf16_h3=False, in_dma="swdge",
                   bufs_sb=3, bufs_obuf=3, ch_sup=8, mask1op=False,
                   mcol_pool=False, out_alt=False, xin_bufs=3, out_split=2,
                   psum2=True)


def _build(weights, reps=1, cfg=None):
    cfg = {**DEFAULT_CFG, **(cfg or {})}
    W1, b1, W2, b2, W3, b3 = weights
    nc = bacc.Bacc("TRN2", target_bir_lowering=False, debug=False,
                   num_devices=NCORES)

    ev = nc.dram_tensor("events", [NPART, FP], F32R, kind="ExternalInput")
    out = nc.dram_tensor("out", [NPART, FOUT], F32, kind="ExternalOutput")

    # Host-preprocessed constant layouts (embedded in the NEFF).
    W1P = np.zeros((128, 128), np.float32)
    for r, c in ((0, 0), (32, 64), (64, 0), (96, 64)):
        W1P[r:r + DIN, c:c + H] = W1
    W2P = np.zeros((128, 128), np.float32)
    W2P[0:64, 0:64] = W2
    W2P[64:128, 64:128] = W2
    W3P = np.zeros((128, 128), np.float32)
    W3P[0:64, 0:64] = W3
    W3P[64:128, 64:128] = W3
    I128 = np.eye(128, dtype=np.float32)
    b1b = np.concatenate([b1, b1])[:, None].astype(np.float32)
    b2b = np.concatenate([b2, b2])[:, None].astype(np.float32)
    b3b = np.concatenate([b3, b3])[:, None].astype(np.float32)

    w1d = nc.inline_tensor(W1P, "w1p")
    w2d = nc.inline_tensor(W2P, "w2p")
    w3d = nc.inline_tensor(W3P, "w3p")
    idd = nc.inline_tensor(I128, "i128")
    b1d = nc.inline_tensor(b1b, "b1b")
    b2d = nc.inline_tensor(b2b, "b2b")
    b3d = nc.inline_tensor(b3b, "b3b")

    with tile.TileContext(nc) as tc:
        _kernel_body(tc, ev, out, w1d, w2d, w3d, idd, b1d, b2d, b3d, reps, cfg)

    nc.compile()
    return nc


def _kernel_body(tc, ev, out, w1d, w2d, w3d, idd, b1d, b2d, b3d, reps, cfg):
    CH_SUP = cfg["ch_sup"]
    CHUNK = SUP * CH_SUP
    NCH = NPART // CHUNK
    BF16 = mybir.dt.bfloat16
    dt_x = BF16 if cfg["bf16_x"] else F32R
    dt_h3 = BF16 if cfg["bf16_h3"] else F32R
    nc = tc.nc
    from contextlib import ExitStack
    ctx = ExitStack()

    cpool = ctx.enter_context(tc.tile_pool(name="consts", bufs=1))
    w1f = cpool.tile([128, 128], F32, name="w1f")
    w2f = cpool.tile([128, 128], F32, name="w2f")
    w3f = cpool.tile([128, 128], F32, name="w3f")
    i1f = cpool.tile([128, 128], F32, name="i1f")
    w1s = cpool.tile([128, 128], dt_x, name="w1s")
    w2s = cpool.tile([128, 128], F32R, name="w2s")
    w3s = cpool.tile([128, 128], F32R, name="w3s")
    i128 = cpool.tile([128, 128], dt_x, name="i128")
    i128b = cpool.tile([128, 128], dt_h3, name="i128b")
    b1s = cpool.tile([128, 1], F32, name="b1s")
    b2s = cpool.tile([128, 1], F32, name="b2s")
    b3s = cpool.tile([128, 1], F32, name="b3s")
    for t, d in ((w1f, w1d), (w2f, w2d), (w3f, w3d), (i1f, idd),
                 (b1s, b1d), (b2s, b2d), (b3s, b3d)):
        nc.sync.dma_start(t[:], d.ap())
    for dst, srct in ((w1s, w1f), (w2s, w2f), (w3s, w3f), (i128, i1f),
                      (i128b, i1f)):
        nc.vector.tensor_copy(dst[:], srct[:])

    bs = cfg["bufs_sb"]
    xpool = ctx.enter_context(tc.tile_pool(name="xin", bufs=cfg["xin_bufs"]))
    xtpool = ctx.enter_context(tc.tile_pool(name="xts", bufs=bs))
    h1pool = ctx.enter_context(tc.tile_pool(name="h1s", bufs=bs))
    h2pool = ctx.enter_context(tc.tile_pool(name="h2s", bufs=bs))
    h3pool = ctx.enter_context(tc.tile_pool(name="h3s", bufs=bs))
    opool = ctx.enter_context(tc.tile_pool(name="obuf", bufs=cfg["bufs_obuf"]))
    tinp = ctx.enter_context(tc.tile_pool(name="tinp", bufs=2, space="PSUM"))
    mmp = ctx.enter_context(
        tc.tile_pool(name="mmp", bufs=2 if cfg["psum2"] else 3, space="PSUM"))
    pbp = ctx.enter_context(tc.tile_pool(name="pbp", bufs=2, space="PSUM")) \
        if cfg["psum2"] else None

    for c in [i for _ in range(reps) for i in range(NCH)]:
        # ---- load [128, 2048] contiguous (64 padded rows per partition)
        xin = xpool.tile([128, CH_SUP * 16 * FP], dt_x, tag="xin")
        src = AP(ev, c * CHUNK * FP, [[CH_SUP * 16 * FP, 128],
                                      [1, CH_SUP * 16 * FP]])
        if cfg["bf16_x"] or cfg["in_dma"] == "swdge":
            nc.gpsimd.dma_start(xin[:], src)
        else:
            nc.scalar.dma_start(xin[:], src)
        obuf = opool.tile([128, CH_SUP * 16 * FOUT], F32, tag="obuf")

        for s in range(CH_SUP):
            xv = xin[:, 512 * s:512 * (s + 1)]
            ov = obuf[:, 1040 * s:1040 * (s + 1)]

            # ---- transpose in: 4 x ([128, 128] window -> [128, 128])
            tpsum = tinp.tile([128, 512], dt_x, tag="tps")
            for t in range(4):
                nc.tensor.transpose(tpsum[:, 128 * t:128 * (t + 1)],
                                    xv[:, 128 * t:128 * (t + 1)], i128[:])

            xts = xtpool.tile([128, 512], dt_x, tag="xts")
            nc.scalar.copy(xts[:], tpsum[:])

            # ---- L1: blockdiag pairs, K=49, N=512
            ps1 = mmp.tile([128, 1024], F32, tag="mm")
            nc.tensor.matmul(ps1[:, 0:512], w1s[0:49, :], xts[0:49, :],
                             start=True, stop=True)
            nc.tensor.matmul(ps1[:, 512:1024], w1s[64:113, :], xts[64:113, :],
                             start=True, stop=True)
            h1s = h1pool.tile([128, 1024], F32R, tag="h1s")
            nc.scalar.activation(h1s[:], ps1[:], AF.Relu, bias=b1s[:])

            # ---- L2
            ps2 = mmp.tile([128, 1024], F32, tag="mm")
            nc.tensor.matmul(ps2[:, 0:512], w2s[:], h1s[:, 0:512],
                             start=True, stop=True)
            nc.tensor.matmul(ps2[:, 512:1024], w2s[:], h1s[:, 512:1024],
                             start=True, stop=True)
            h2s = h2pool.tile([128, 1024], F32R, tag="h2s")
            nc.vector.tensor_scalar(h2s[:], ps2[:], b2s[:], 0.0,
                                    ALU.add, ALU.max)

            # ---- L3
            ps3 = mmp.tile([128, 1024], F32, tag="mm")
            nc.tensor.matmul(ps3[:, 0:512], w3s[:], h2s[:, 0:512],
                             start=True, stop=True)
            nc.tensor.matmul(ps3[:, 512:1024], w3s[:], h2s[:, 512:1024],
                             start=True, stop=True)
            h3s = h3pool.tile([128, 1024], dt_h3, tag="h3s")
            nc.scalar.activation(h3s[:], ps3[:], AF.Relu, bias=b3s[:])

            # ---- transpose out: 8 x [128,128] -> particle-major pairs
            if cfg["psum2"]:
                pbt = [pbp.tile([128, 512], dt_h3, tag="pb", name=f"pb{hh}")
                       for hh in range(2)]
                for u in range(8):
                    nc.tensor.transpose(pbt[u // 4][:, 128 * (u % 4):128 * (u % 4 + 1)],
                                        h3s[:, 128 * u:128 * (u + 1)], i128b[:])
                psb_half = lambda hh: pbt[hh][:]
            else:
                psb = mmp.tile([128, 1024], dt_h3, tag="mm")
                for u in range(8):
                    nc.tensor.transpose(psb[:, 128 * u:128 * (u + 1)],
                                        h3s[:, 128 * u:128 * (u + 1)], i128b[:])
                psb_half = lambda hh: psb[:, 512 * hh:512 * (hh + 1)]

            # ---- masked copy into output buffer + mask column
            # column groups: half h=0 -> blocks (4t, 4t+1); h=1 -> (4t+2, 4t+3)
            if cfg["mask1op"]:
                d0 = ov[:, 0:64]
                dst = AP(d0.tensor, d0.offset,
                         [list(d0.ap[0]), [130, 2], [260, 4], [65, 2], [1, 64]])
                m0 = xv[:, 16:17]
                msk = AP(m0.tensor, m0.offset,
                         [list(m0.ap[0]), [64, 2], [128, 4], [32, 2], [0, 64]])
                raise NotImplementedError("mask1op unsupported")
            else:
                for h in range(2):
                    dst = _inject(ov[:, 130 * h:130 * h + 64],
                                  [[260, 4], [65, 2]])
                    msk = xv[:, 16 + 64 * h:16 + 64 * h + 1]
                    msk = AP(msk.tensor, msk.offset,
                             [list(msk.ap[0]), [128, 4], [32, 2], [0, 64]])
                    nc.vector.tensor_tensor(dst, psb_half(h), msk, ALU.mult)
            mcol_dst = _inject(ov[:, 64:65], [[65, 16]])
            mcol_src = _inject(xv[:, 16:17], [[32, 16]])
            if cfg["mcol_pool"]:
                nc.gpsimd.tensor_copy(mcol_dst, mcol_src)
            else:
                nc.vector.tensor_copy(mcol_dst, mcol_src)

        oeng = nc.scalar if (cfg["out_alt"] and c % 2 == 1) else nc.sync
        osp = cfg["out_split"]
        sub = CH_SUP // osp
        for j in range(osp):
            # partition p holds rows c*CHUNK + p*(CH_SUP*16) + [0, CH_SUP*16);
            # sub-chunk j is the per-partition column slice [j*sub*16, ...)
            dst = AP(out, c * CHUNK * FOUT + j * sub * 16 * FOUT,
                     [[CH_SUP * 16 * FOUT, 128], [1, sub * 16 * FOUT]])
            oeng.dma_start(dst, obuf[:, j * sub * 1040:(j + 1) * sub * 1040])

    ctx.close()


def kernel(events, W1, b1, W2, b2, W3, b3):
    global LAST_RESULTS
    events = np.ascontiguousarray(np.asarray(events, dtype=np.float32))
    ws = tuple(np.ascontiguousarray(np.asarray(a, np.float32))
               for a in (W1, b1, W2, b2, W3, b3))
    key = tuple(a.tobytes() for a in ws)  # weights are baked into the NEFF
    if _CACHE.get("key") != key:
        _CACHE["nc"] = _build(ws, cfg=CFG)
        _CACHE["key"] = key
    nc = _CACHE["nc"]

    flat = events.reshape(B * P, F)
    padded = np.zeros((B * P, FP), dtype=np.float32)
    padded[:, :F] = flat
    in_maps = [{"events": np.ascontiguousarray(padded[c * NPART:(c + 1) * NPART])}
               for c in range(NCORES)]
    res = run_bass_kernel_spmd(nc, in_maps, core_ids=list(range(NCORES)),
                               trace=TRACE)
    LAST_RESULTS = res
    out = np.concatenate([res.results[c]["out"] for c in range(NCORES)], axis=0)
    return out.reshape(B, P, FOUT)
